# revision 20
# baseline (speedup 1.0000x reference)
"""Multi-head attention (B=4, S=2048, D=512, H=8) on 8 TRN2 NeuronCores.

Sharding: core c handles batch b = c//2 and query-half q = c%2 (1024 query
rows). Attention needs all keys/values of the batch, so K/V work is
duplicated between the two cores of a batch pair; no cross-core
communication. Host marshalling transposes inputs to [d_in, s] and splits
x and 16*W_{q,k,v} into fp8e4m3 hi/lo pairs (hi = fp8(x), lo = fp8(x-hi));
W_out and the transpose identity ship as bf16. Output returns bf16 and is
widened on the host.

Per-core dataflow:
  1. q/k/v projections run as fp8 DoubleRow matmuls over the three hi/lo
     cross terms (hi*hi + hi*lo + lo*hi; the lo*lo term is ~0.4% of one ulp
     and is dropped). Six 0.5-cyc/row matmuls replace four 1-cyc/row fp32
     chunk matmuls (25% fewer PE cycles) at ~0.4% worst-case error instead
     of fp8's 6%. qT/kT land head-major [d_out, s] fp32; v lands natural
     [s, d_out] and is scattered to a bf16 [s%128, blk, 8*65] "augmented"
     tile whose 16.0 column per head makes PV also emit the softmax
     denominator (16 cancels the 16x weight scaling).
  2. Flat pipeline over (head, key-block) slots: scores st = kT.T @ qT in
     PSUM (fp32r); ACT computes pT = exp(st/2048) straight to bf16 (1/2048
     folds 1/sqrt(DH) and the two 16x weight scalings). ACT does nothing
     but the 128 exps -- it is the critical path (~134us); PE (~122us)
     hides everything else in the slot gaps. PV runs transposed: per
     (head, block, q-chunk) out[q,d] = pT-chunk.T @ v_aug-slice, making v
     (65 cols) the moving operand -- 8x fewer PE rows than the [d,q]
     orientation -- accumulating [128q, 8qc, 65] in PSUM across blocks.
  3. Per head: denominator sits at column 64 per q-chunk, so normalize is
     a per-partition reciprocal + 8 tensor_scalar multiplies (no gpsimd
     broadcast), writing natural-layout bf16 attention output.
  4. Per head-pair: PE transposes the finished 128-column band (via
     identity matmul), and the W_out contribution is accumulated into SBUF
     via DVE adds, so only the last pair's matmuls sit in the tail.
     Output written bf16, DMA'd per 128-row block as it completes.

PSUM (8 banks): st 2x2 + pv 2 + pp (proj/transpose/out-proj) 2.
"""

import numpy as np
import ml_dtypes

import concourse.bass as bass
from concourse import bacc
import concourse.mybir as mybir
import concourse.tile as tile
from concourse.bass_utils import run_bass_kernel_spmd

B, S, D, H = 4, 2048, 512, 8
DH = D // H          # 64
P = 128
SQ = S // 2          # 1024 query rows per core
NCORES = 8
NKB = S // P         # 16 key blocks
VW = DH + 1          # 65: per-head v columns + denominator column
F32 = mybir.dt.float32
F32R = mybir.dt.float32r
F8 = mybir.dt.float8e4
BF16 = mybir.dt.bfloat16
DR = mybir.MatmulPerfMode.DoubleRow
EXP = mybir.ActivationFunctionType.Exp
FP8 = ml_dtypes.float8_e4m3
BF16NP = ml_dtypes.bfloat16

WSCALE = 16.0                          # host scales W_{q,k,v} by this
EXP_SCALE = 0.125 / (WSCALE * WSCALE)  # 1/2048: undo 16*16, apply 1/sqrt(DH)


def _r(ap):
    return ap.bitcast(F32R)


def _build_mha(tc, out_d, xq_d, xk_d, xv_d, w_ds, wo_d, iden_d):
    nc = tc.nc

    with (
        tc.tile_pool(name="consts", bufs=1) as cpool,
        tc.tile_pool(name="work", bufs=2) as wpool,
    ):
        # ---------- static SBUF ----------
        wsrc = cpool.tile([P, 64], F32)
        nc.gpsimd.memset(wsrc, 0.0)
        warm = cpool.tile([P, 1], F32)
        # dummy exp pulls the ACT exp-table load to t=0
        nc.scalar.activation(warm, wsrc[:, 0:1], EXP)

        xq8 = [cpool.tile([P, 4, SQ], F8, name=f"xq8_{i}") for i in range(2)]
        xk8 = [cpool.tile([P, 4, S], F8, name=f"xk8_{i}") for i in range(2)]
        xv8 = [cpool.tile([P, 4, S], F8, name=f"xv8_{i}") for i in range(2)]
        w8 = {}  # (which, hi/lo) -> [128, 4, D] fp8
        for which in ("q", "k", "v"):
            for i in range(2):
                w8[which, i] = cpool.tile([P, 4, D], F8, name=f"w8{which}{i}")
        wo_sb = cpool.tile([P, 4, D], BF16)
        iden = cpool.tile([P, P], BF16)
        qT = cpool.tile([P, 4, SQ], BF16)    # [d_out%128, d_out//128, s]
        kT = cpool.tile([P, 4, S], BF16)
        v_aug = cpool.tile([P, NKB, H * VW], BF16)
        att_n = cpool.tile([P, 8, D], BF16)  # [q%128, q//128, d] normalized
        outT = cpool.tile([P, 4, SQ], BF16)  # [d%128, d//128, s]
        obacc = cpool.tile([P, 8, D], F32)   # out-proj accumulator per s-block
        obf = cpool.tile([P, 8, D], BF16)

        # ---------- DMA ----------
        # The sim serializes all DMA on one ~360GB/s resource, so global
        # order matters: the first-exp chain (wq m0, xq, wk m0, xk n0) goes
        # first; everything else is needed slots-to-heads later.
        wd = {}
        for wi, (which, i) in enumerate([(w, i) for w in "qkv" for i in range(2)]):
            wd[which, i] = w_ds[wi]
        # No DMA ever issues from the scalar queue: each issue occupies the
        # ACT sequencer for ~667ns (and blocks on the HWDGE ring), which
        # would push the whole exp stream out by many microseconds.
        # sync: the first-exp chain in exact need order, then the k stream
        def sync2(dst, src):
            for i in range(2):
                nc.sync.dma_start(dst[i], src[i])

        def xpiece(x8, x_d, n):
            sync2([x8[i][:, :, n * 512 : (n + 1) * 512] for i in range(2)],
                  [x_d[i][:, :, n * 512 : (n + 1) * 512] for i in range(2)])

        sync2([w8["q", i][:, :, 0:P] for i in range(2)],
              [wd["q", i][:, :, 0:P] for i in range(2)])
        xpiece(xq8, xq_d, 0)
        sync2([w8["k", i][:, :, 0:P] for i in range(2)],
              [wd["k", i][:, :, 0:P] for i in range(2)])
        xpiece(xk8, xk_d, 0)
        xpiece(xq8, xq_d, 1)
        xpiece(xk8, xk_d, 1)
        sync2([w8["v", i] for i in range(2)], [wd["v", i] for i in range(2)])
        xpiece(xv8, xv_d, 0)
        xpiece(xk8, xk_d, 2)
        xpiece(xv8, xv_d, 1)
        xpiece(xk8, xk_d, 3)
        xpiece(xv8, xv_d, 2)
        xpiece(xv8, xv_d, 3)
        # gpsimd: later-needed tensors. The big obacc memset first: it holds
        # the Pool engine ~3.4us so these issues (and their transfers) stay
        # out of the critical early window of the serialized DMA resource.
        nc.gpsimd.memset(obacc, 0.0)
        nc.gpsimd.dma_start(iden, iden_d)
        for i in range(2):
            nc.gpsimd.dma_start(w8["q", i][:, :, P:D], wd["q", i][:, :, P:D])
            nc.gpsimd.dma_start(w8["k", i][:, :, P:D], wd["k", i][:, :, P:D])
        nc.gpsimd.dma_start(wo_sb, wo_d)

        # denominator column of v_aug = 16.0 (cancels the 16x W_v scaling)
        nc.gpsimd.memset(
            v_aug.rearrange("p n (h e) -> p n h e", e=VW)[:, :, :, DH:VW], WSCALE
        )

        # ---------- PSUM pools (8 banks exactly) ----------
        with (
            tc.tile_pool(name="ps_st", bufs=2, space="PSUM") as ps_st,
            tc.tile_pool(name="ps_pv", bufs=2, space="PSUM") as ps_pv,
            tc.tile_pool(name="ps_pp", bufs=2, space="PSUM") as ps_pp,
        ):
            # hi/lo cross terms: (w_hi,x_hi), (w_hi,x_lo), (w_lo,x_hi)
            TERMS = ((0, 0), (0, 1), (1, 0))

            def proj_qk(which, dst, m, ns, alloc=None, copy_eng=None):
                """project d_out chunk m for s-chunks ns -> dst[:, m, :]."""
                x8 = xq8 if which == "q" else xk8
                for n in ns:
                    if alloc is None:
                        pp = ps_pp.tile([P, 512], F32, tag="pp", name="pp_qk")
                    else:
                        pp = alloc()
                    for ti, (wi, xi) in enumerate(TERMS):
                        for j in range(2):
                            nc.tensor.matmul(
                                pp,
                                w8[which, wi][:, 2 * j : 2 * j + 2, m * P : (m + 1) * P],
                                x8[xi][:, 2 * j : 2 * j + 2, n * 512 : (n + 1) * 512],
                                start=(ti == 0 and j == 0),
                                stop=(ti == 2 and j == 1),
                                perf_mode=DR,
                            )
                    if copy_eng is None:
                        nc.vector.tensor_copy(dst[:, m, n * 512 : (n + 1) * 512], pp)
                    else:
                        copy_eng(dst[:, m, n * 512 : (n + 1) * 512], pp)
                    yield

            def proj_v(n):
                """v projection for s chunk n (4 key blocks), natural layout."""
                for sb in range(4):
                    pp = ps_pp.tile([P, 512], F32, tag="pp", name="pp_v")
                    for ti, (wi, xi) in enumerate(TERMS):
                        for j in range(2):
                            nc.tensor.matmul(
                                pp,
                                xv8[xi][
                                    :, 2 * j : 2 * j + 2,
                                    n * 512 + sb * P : n * 512 + (sb + 1) * P,
                                ],
                                w8["v", wi][:, 2 * j : 2 * j + 2, :],
                                start=(ti == 0 and j == 0),
                                stop=(ti == 2 and j == 1),
                                perf_mode=DR,
                            )
                    nc.vector.tensor_copy(
                        v_aug.rearrange("p n (h e) -> p n h e", e=VW)[
                            :, n * 4 + sb, :, 0:DH
                        ],
                        pp.rearrange("p (h d) -> p h d", d=DH),
                    )
                    yield

            def outproj_nb(hp, nb):
                """transpose + W_out contribution of head-pair hp, s-block nb."""
                last = hp == 3
                tpp = ps_pp.tile([P, 512], F32, tag="pp", name="tp")
                tp = tpp.bitcast(BF16)[:, 0:P]
                nc.tensor.transpose(tp, att_n[:, nb, hp * P : (hp + 1) * P], iden)
                dst = outT[:, hp, nb * P : (nb + 1) * P]
                if last:
                    nc.scalar.copy(dst, tp)  # ACT is idle post-stream
                else:
                    nc.vector.tensor_copy(dst, tp)
                pf = ps_pp.tile([P, 512], F32, tag="pp", name="pf")
                nc.tensor.matmul(
                    pf,
                    outT[:, hp, nb * P : (nb + 1) * P],
                    wo_sb[:, hp, :],
                    start=True,
                    stop=True,
                )
                if hp == 0:
                    nc.vector.tensor_copy(obacc[:, nb, :], pf)
                elif not last:
                    nc.vector.tensor_add(obacc[:, nb, :], obacc[:, nb, :], pf)
                else:
                    nc.vector.tensor_add(obf[:, nb, :], obacc[:, nb, :], pf)
                    nc.sync.dma_start(out_d[nb * P : (nb + 1) * P, :], obf[:, nb, :])
                yield

            def outproj_hp(hp):
                for nb in range(8):
                    yield from outproj_nb(hp, nb)

            # PE p-state warm-up: the ramp model needs ~3us of continuous
            # execution before the PE hits full clock; burn it on dummy
            # matmuls while the first DMA pieces land.
            for _ in range(40):
                ppd = ps_pp.tile([P, 512], F32, tag="pp", name="ppd")
                nc.tensor.matmul(
                    ppd[0:1, 0:64], _r(wsrc[:, 0:1]), _r(wsrc), start=True, stop=True
                )

            # head 0 needs q chunk 0 (both s-chunks) + k chunk 0 block 0.
            # Spread the three groups over distinct PSUM buffers (borrowing
            # idle st-pool banks) so they don't serialize on pool reuse.
            def st_half():
                return ps_st.tile([P, SQ], F32, tag="st", name="pp_pre")[:, 0:512]
            # pre-stream copies ride the idle ACT engine (it has nothing to
            # do before the first exp; DVE would add queue latency)
            for _ in proj_qk("q", qT, 0, (0,), alloc=st_half, copy_eng=nc.scalar.copy):
                pass
            for _ in proj_qk("k", kT, 0, (0,), alloc=st_half, copy_eng=nc.scalar.copy):
                pass
            for _ in proj_qk("q", qT, 0, (1,), copy_eng=nc.scalar.copy):
                pass

            # Three priority queues, one task-step per slot: pv units on even
            # slots, projections on odd slots, out-proj fills leftover slots.
            # qkq order honors data deadlines: k m0 rest for head-0 blocks
            # 4..15, v chunks for the trailing PV, then later m chunks.
            vq = [
                proj_qk("k", kT, 0, (1,)),
                proj_v(0),
                proj_qk("k", kT, 0, (2,)),
                proj_v(1),
                proj_qk("k", kT, 0, (3,)),
                proj_v(2),
                proj_v(3),
            ]
            qkq = []
            for m in (1, 2, 3):
                qkq.append(proj_qk("q", qT, m, (0, 1)))
                qkq.append(proj_qk("k", kT, m, (0, 1, 2, 3)))
            pvq = []
            opq = []

            def step_q(q):
                while q:
                    if next(q[0], "done") == "done":
                        q.pop(0)
                    else:
                        return True
                return False

            def step_tasks():
                for q in (vq, pvq, qkq, opq):
                    if step_q(q):
                        return

            # ---------- attention slot stream ----------
            # pT ring: 24 tiles so exp(h+1, blk) only reuses a tile whose
            # PV reads (running during head h+1, one qc-unit per slot,
            # done by slot ~7) have completed.
            PT_BUFS = 32

            def pv_head(h, pTs):
                """PV + normalize for head h: per q-chunk, accumulate all 16
                key blocks in one PSUM bank (one zero region), then divide by
                the denominator column. For the last head, the head-pair-3
                transpose/out-proj steps are inlined per q-chunk to shorten
                the tail."""
                for qc in range(8):
                    pv = ps_pv.tile([P, 512], F32, tag="pv", name="pv")
                    for blk in range(NKB):
                        nc.tensor.matmul(
                            pv[:, 0:VW],
                            pTs[blk][:, qc * P : (qc + 1) * P],
                            v_aug[:, blk, h * VW : (h + 1) * VW],
                            start=(blk == 0),
                            stop=(blk == NKB - 1),
                        )
                    recip = wpool.tile([P, 1], F32, tag="recip", bufs=2)
                    nc.vector.reciprocal(recip, pv[:, DH : DH + 1])
                    if h == H - 1:
                        # ACT is idle post-stream; keep the tail off DVE
                        nc.scalar.mul(
                            att_n[:, qc, h * DH : (h + 1) * DH], pv[:, 0:DH], recip
                        )
                    else:
                        nc.vector.tensor_scalar_mul(
                            att_n[:, qc, h * DH : (h + 1) * DH], pv[:, 0:DH], recip
                        )
                    if h == H - 1:
                        for _ in outproj_nb(3, qc):
                            pass
                    yield
                if h % 2 == 1 and h < H - 1:
                    opq.append(outproj_hp(h // 2))

            for h in range(H):
                po = (h % 2) * DH
                mc = h // 2
                pTs = []
                for blk in range(NKB):
                    # scores + exp go first so a stalled task matmul (waiting
                    # on PSUM reuse) never blocks them in the in-order PE queue
                    st = ps_st.tile([P, SQ], F32, tag="st")
                    for nq in range(2):
                        nc.tensor.matmul(
                            st[:, nq * 512 : (nq + 1) * 512],
                            kT[po : po + DH, mc, blk * P : (blk + 1) * P],
                            qT[po : po + DH, mc, nq * 512 : (nq + 1) * 512],
                            start=True,
                            stop=True,
                        )
                    pT = wpool.tile([P, SQ], BF16, tag="pT", bufs=PT_BUFS)
                    if h == 0 and blk == 0:
                        # two half-width exps: the first only needs q chunk 0
                        for nq in range(2):
                            nc.scalar.activation(
                                pT[:, nq * 512 : (nq + 1) * 512],
                                st[:, nq * 512 : (nq + 1) * 512],
                                EXP,
                                scale=float(EXP_SCALE),
                            )
                    else:
                        nc.scalar.activation(pT, st, EXP, scale=float(EXP_SCALE))
                    pTs.append(pT)
                    if h == 0:
                        step_q(vq)
                    else:
                        step_tasks()
                pvq.append(pv_head(h, pTs))
            for q in (vq, pvq, opq, qkq):
                while step_q(q):
                    pass


_CACHED_NC = None


def _get_nc():
    global _CACHED_NC
    if _CACHED_NC is not None:
        return _CACHED_NC
    nc = bacc.Bacc("TRN2", target_bir_lowering=False, debug=False)
    def dt(name, shape, dtype):
        return nc.dram_tensor(name, shape, dtype, kind="ExternalInput").ap()
    xq = [dt(f"xq8_{i}", [P, 4, SQ], F8) for i in range(2)]
    xk = [dt(f"xk8_{i}", [P, 4, S], F8) for i in range(2)]
    xv = [dt(f"xv8_{i}", [P, 4, S], F8) for i in range(2)]
    w_ds = [dt(f"w8{w}{i}", [P, 4, D], F8) for w in "qkv" for i in range(2)]
    wo = dt("wo", [P, 4, D], BF16)
    iden = dt("iden", [P, P], BF16)
    out = nc.dram_tensor("out", [SQ, D], BF16, kind="ExternalOutput").ap()
    with tile.TileContext(nc) as tc:
        _build_mha(tc, out, xq, xk, xv, w_ds, wo, iden)
    nc.compile()
    return_nc = nc
    globals()["_CACHED_NC"] = return_nc
    return return_nc


def _cwise(xT):
    # [512, n] -> [128, 4, n] (d_in chunked)
    n = xT.shape[1]
    return np.ascontiguousarray(xT.reshape(4, P, n).transpose(1, 0, 2))


def _split8(a):
    hi = a.astype(FP8)
    lo = (a - hi.astype(np.float32)).astype(FP8)
    return hi, lo


def _w_arrange(w):
    return _cwise(np.asarray(w, np.float32) * WSCALE)


def _run(in_query, in_key, in_value, W_q, W_k, W_v, W_out, **run_kwargs):
    f = lambda a: np.asarray(a, dtype=np.float32)
    in_query, in_key, in_value = f(in_query), f(in_key), f(in_value)
    w_splits = []
    for w in (W_q, W_k, W_v):
        w_splits.extend(_split8(_w_arrange(w)))
    wo = _cwise(f(W_out)).astype(BF16NP)
    iden = np.eye(P, dtype=BF16NP)
    xk8 = [_split8(_cwise(in_key[b].T)) for b in range(B)]
    xv8 = [_split8(_cwise(in_value[b].T)) for b in range(B)]
    in_maps = []
    for c in range(NCORES):
        b, half = c // 2, c % 2
        xq_hi, xq_lo = _split8(_cwise(in_query[b, half * SQ : (half + 1) * SQ, :].T))
        im = {"xq8_0": xq_hi, "xq8_1": xq_lo,
              "xk8_0": xk8[b][0], "xk8_1": xk8[b][1],
              "xv8_0": xv8[b][0], "xv8_1": xv8[b][1],
              "wo": wo, "iden": iden}
        for wi, (w, i) in enumerate([(w, i) for w in "qkv" for i in range(2)]):
            im[f"w8{w}{i}"] = w_splits[wi]
        in_maps.append(im)
    res = run_bass_kernel_spmd(_get_nc(), in_maps, list(range(NCORES)), **run_kwargs)
    out = np.empty((B, S, D), np.float32)
    for c in range(NCORES):
        b, half = c // 2, c % 2
        out[b, half * SQ : (half + 1) * SQ, :] = res.results[c]["out"].astype(
            np.float32
        )
    return out, res


def kernel(in_query, in_key, in_value, W_q, W_k, W_v, W_out):
    out, _ = _run(in_query, in_key, in_value, W_q, W_k, W_v, W_out)
    return out


# revision 21
# speedup vs baseline: 1.0174x; 1.0174x over previous
"""Multi-head attention (B=4, S=2048, D=512, H=8) on 8 TRN2 NeuronCores.

Sharding: core c handles batch b = c//2 and query-half q = c%2 (1024 query
rows). Attention needs all keys/values of the batch, so K/V work is
duplicated between the two cores of a batch pair; no cross-core
communication. Host marshalling transposes inputs to [d_in, s] and splits
x and 16*W_{q,k,v} into fp8e4m3 hi/lo pairs (hi = fp8(x), lo = fp8(x-hi));
W_out and the transpose identity ship as bf16. Output returns bf16 and is
widened on the host.

Per-core dataflow:
  1. q/k/v projections run as fp8 DoubleRow matmuls over the three hi/lo
     cross terms (hi*hi + hi*lo + lo*hi; the lo*lo term is ~0.4% of one ulp
     and is dropped). Six 0.5-cyc/row matmuls replace four 1-cyc/row fp32
     chunk matmuls (25% fewer PE cycles) at ~0.4% worst-case error instead
     of fp8's 6%. qT/kT land head-major [d_out, s] fp32; v lands natural
     [s, d_out] and is scattered to a bf16 [s%128, blk, 8*65] "augmented"
     tile whose 16.0 column per head makes PV also emit the softmax
     denominator (16 cancels the 16x weight scaling).
  2. Flat pipeline over (head, key-block) slots: scores st = kT.T @ qT in
     PSUM (fp32r); ACT computes pT = exp(st/2048) straight to bf16 (1/2048
     folds 1/sqrt(DH) and the two 16x weight scalings). ACT does nothing
     but the 128 exps -- it is the critical path (~134us); PE (~122us)
     hides everything else in the slot gaps. PV runs transposed: per
     (head, block, q-chunk) out[q,d] = pT-chunk.T @ v_aug-slice, making v
     (65 cols) the moving operand -- 8x fewer PE rows than the [d,q]
     orientation -- accumulating [128q, 8qc, 65] in PSUM across blocks.
  3. Per head: denominator sits at column 64 per q-chunk, so normalize is
     a per-partition reciprocal + 8 tensor_scalar multiplies (no gpsimd
     broadcast), writing natural-layout bf16 attention output.
  4. Per head-pair: PE transposes the finished 128-column band (via
     identity matmul), and the W_out contribution is accumulated into SBUF
     via DVE adds, so only the last pair's matmuls sit in the tail.
     Output written bf16, DMA'd per 128-row block as it completes.

PSUM (8 banks): st 2x2 + pv 2 + pp (proj/transpose/out-proj) 2.
"""

import numpy as np
import ml_dtypes

import concourse.bass as bass
from concourse import bacc
import concourse.mybir as mybir
import concourse.tile as tile
from concourse.bass_utils import run_bass_kernel_spmd

B, S, D, H = 4, 2048, 512, 8
DH = D // H          # 64
P = 128
SQ = S // 2          # 1024 query rows per core
NCORES = 8
NKB = S // P         # 16 key blocks
VW = DH + 1          # 65: per-head v columns + denominator column
F32 = mybir.dt.float32
F32R = mybir.dt.float32r
F8 = mybir.dt.float8e4
BF16 = mybir.dt.bfloat16
DR = mybir.MatmulPerfMode.DoubleRow
EXP = mybir.ActivationFunctionType.Exp
FP8 = ml_dtypes.float8_e4m3
BF16NP = ml_dtypes.bfloat16

WSCALE = 16.0                          # host scales W_{q,k,v} by this
EXP_SCALE = 0.125 / (WSCALE * WSCALE)  # 1/2048: undo 16*16, apply 1/sqrt(DH)


def _r(ap):
    return ap.bitcast(F32R)


def _build_mha(tc, out_d, xq_d, xk_d, xv_d, w_ds, wo_d, iden_d):
    nc = tc.nc

    with (
        tc.tile_pool(name="consts", bufs=1) as cpool,
        tc.tile_pool(name="work", bufs=2) as wpool,
    ):
        # ---------- static SBUF ----------
        wsrc = cpool.tile([P, 64], F32)
        nc.gpsimd.memset(wsrc, 0.0)
        warm = cpool.tile([P, 1], F32)
        # dummy exp pulls the ACT exp-table load to t=0
        nc.scalar.activation(warm, wsrc[:, 0:1], EXP)

        xq8 = [cpool.tile([P, 4, SQ], F8, name=f"xq8_{i}") for i in range(2)]
        xk8 = [cpool.tile([P, 4, S], F8, name=f"xk8_{i}") for i in range(2)]
        xv8 = [cpool.tile([P, 4, S], F8, name=f"xv8_{i}") for i in range(2)]
        w8 = {}  # (which, hi/lo) -> [128, 4, D] fp8
        for which in ("q", "k", "v"):
            for i in range(2):
                w8[which, i] = cpool.tile([P, 4, D], F8, name=f"w8{which}{i}")
        wo_sb = cpool.tile([P, 4, D], BF16)
        iden = cpool.tile([P, P], BF16)
        qT = cpool.tile([P, 4, SQ], BF16)    # [d_out%128, d_out//128, s]
        kT = cpool.tile([P, 4, S], BF16)
        v_aug = cpool.tile([P, NKB, H * VW], BF16)
        att_n = cpool.tile([P, 8, D], BF16)  # [q%128, q//128, d] normalized
        outT = cpool.tile([P, 4, SQ], BF16)  # [d%128, d//128, s]
        obacc = cpool.tile([P, 8, D], F32)   # out-proj accumulator per s-block
        obf = cpool.tile([P, 8, D], BF16)

        # ---------- DMA ----------
        # The sim serializes all DMA on one ~360GB/s resource, so global
        # order matters: the first-exp chain (wq m0, xq, wk m0, xk n0) goes
        # first; everything else is needed slots-to-heads later.
        wd = {}
        for wi, (which, i) in enumerate([(w, i) for w in "qkv" for i in range(2)]):
            wd[which, i] = w_ds[wi]
        # No DMA ever issues from the scalar queue: each issue occupies the
        # ACT sequencer for ~667ns (and blocks on the HWDGE ring), which
        # would push the whole exp stream out by many microseconds.
        # sync: the first-exp chain in exact need order, then the k stream
        def sync2(dst, src):
            for i in range(2):
                nc.sync.dma_start(dst[i], src[i])

        def xpiece(x8, x_d, n):
            sync2([x8[i][:, :, n * 512 : (n + 1) * 512] for i in range(2)],
                  [x_d[i][:, :, n * 512 : (n + 1) * 512] for i in range(2)])

        sync2([w8["q", i][:, :, 0:P] for i in range(2)],
              [wd["q", i][:, :, 0:P] for i in range(2)])
        xpiece(xq8, xq_d, 0)
        sync2([w8["k", i][:, :, 0:P] for i in range(2)],
              [wd["k", i][:, :, 0:P] for i in range(2)])
        xpiece(xk8, xk_d, 0)
        xpiece(xq8, xq_d, 1)
        xpiece(xk8, xk_d, 1)
        sync2([w8["v", i] for i in range(2)], [wd["v", i] for i in range(2)])
        xpiece(xv8, xv_d, 0)
        xpiece(xk8, xk_d, 2)
        xpiece(xv8, xv_d, 1)
        xpiece(xk8, xk_d, 3)
        xpiece(xv8, xv_d, 2)
        xpiece(xv8, xv_d, 3)
        sync2([w8["q", i][:, :, P:D] for i in range(2)],
              [wd["q", i][:, :, P:D] for i in range(2)])
        sync2([w8["k", i][:, :, P:D] for i in range(2)],
              [wd["k", i][:, :, P:D] for i in range(2)])
        # gpsimd: later-needed tensors. The big obacc memset first: it holds
        # the Pool engine ~3.4us so these issues (and their transfers) stay
        # out of the critical early window of the serialized DMA resource.
        nc.gpsimd.memset(obacc, 0.0)
        nc.gpsimd.dma_start(iden, iden_d)
        nc.gpsimd.dma_start(wo_sb, wo_d)

        # denominator column of v_aug = 16.0 (cancels the 16x W_v scaling)
        nc.gpsimd.memset(
            v_aug.rearrange("p n (h e) -> p n h e", e=VW)[:, :, :, DH:VW], WSCALE
        )

        # ---------- PSUM pools (8 banks exactly) ----------
        with (
            tc.tile_pool(name="ps_st", bufs=2, space="PSUM") as ps_st,
            tc.tile_pool(name="ps_pv", bufs=2, space="PSUM") as ps_pv,
            tc.tile_pool(name="ps_pp", bufs=2, space="PSUM") as ps_pp,
        ):
            # hi/lo cross terms: (w_hi,x_hi), (w_hi,x_lo), (w_lo,x_hi)
            TERMS = ((0, 0), (0, 1), (1, 0))

            def proj_qk(which, dst, m, ns, alloc=None, copy_eng=None):
                """project d_out chunk m for s-chunks ns -> dst[:, m, :]."""
                x8 = xq8 if which == "q" else xk8
                for n in ns:
                    if alloc is None:
                        pp = ps_pp.tile([P, 512], F32, tag="pp", name="pp_qk")
                    else:
                        pp = alloc()
                    for ti, (wi, xi) in enumerate(TERMS):
                        for j in range(2):
                            nc.tensor.matmul(
                                pp,
                                w8[which, wi][:, 2 * j : 2 * j + 2, m * P : (m + 1) * P],
                                x8[xi][:, 2 * j : 2 * j + 2, n * 512 : (n + 1) * 512],
                                start=(ti == 0 and j == 0),
                                stop=(ti == 2 and j == 1),
                                perf_mode=DR,
                            )
                    if copy_eng is None:
                        nc.vector.tensor_copy(dst[:, m, n * 512 : (n + 1) * 512], pp)
                    else:
                        copy_eng(dst[:, m, n * 512 : (n + 1) * 512], pp)
                    yield

            def proj_v(n):
                """v projection for s chunk n (4 key blocks), natural layout."""
                for sb in range(4):
                    pp = ps_pp.tile([P, 512], F32, tag="pp", name="pp_v")
                    for ti, (wi, xi) in enumerate(TERMS):
                        for j in range(2):
                            nc.tensor.matmul(
                                pp,
                                xv8[xi][
                                    :, 2 * j : 2 * j + 2,
                                    n * 512 + sb * P : n * 512 + (sb + 1) * P,
                                ],
                                w8["v", wi][:, 2 * j : 2 * j + 2, :],
                                start=(ti == 0 and j == 0),
                                stop=(ti == 2 and j == 1),
                                perf_mode=DR,
                            )
                    nc.vector.tensor_copy(
                        v_aug.rearrange("p n (h e) -> p n h e", e=VW)[
                            :, n * 4 + sb, :, 0:DH
                        ],
                        pp.rearrange("p (h d) -> p h d", d=DH),
                    )
                    yield

            def outproj_nb(hp, nb):
                """transpose + W_out contribution of head-pair hp, s-block nb."""
                last = hp == 3
                tpp = ps_pp.tile([P, 512], F32, tag="pp", name="tp")
                tp = tpp.bitcast(BF16)[:, 0:P]
                nc.tensor.transpose(tp, att_n[:, nb, hp * P : (hp + 1) * P], iden)
                dst = outT[:, hp, nb * P : (nb + 1) * P]
                if last:
                    nc.scalar.copy(dst, tp)  # ACT is idle post-stream
                else:
                    nc.vector.tensor_copy(dst, tp)
                pf = ps_pp.tile([P, 512], F32, tag="pp", name="pf")
                nc.tensor.matmul(
                    pf,
                    outT[:, hp, nb * P : (nb + 1) * P],
                    wo_sb[:, hp, :],
                    start=True,
                    stop=True,
                )
                if hp == 0:
                    nc.vector.tensor_copy(obacc[:, nb, :], pf)
                elif not last:
                    nc.vector.tensor_add(obacc[:, nb, :], obacc[:, nb, :], pf)
                else:
                    nc.vector.tensor_add(obf[:, nb, :], obacc[:, nb, :], pf)
                    nc.sync.dma_start(out_d[nb * P : (nb + 1) * P, :], obf[:, nb, :])
                yield

            def outproj_hp(hp):
                for nb in range(8):
                    yield from outproj_nb(hp, nb)

            # PE p-state warm-up: the ramp model needs ~3us of continuous
            # execution before the PE hits full clock; burn it on dummy
            # matmuls while the first DMA pieces land.
            for _ in range(40):
                ppd = ps_pp.tile([P, 512], F32, tag="pp", name="ppd")
                nc.tensor.matmul(
                    ppd[0:1, 0:64], _r(wsrc[:, 0:1]), _r(wsrc), start=True, stop=True
                )

            # head 0 needs q chunk 0 (both s-chunks) + k chunk 0 block 0.
            # Spread the three groups over distinct PSUM buffers (borrowing
            # idle st-pool banks) so they don't serialize on pool reuse.
            def st_half():
                return ps_st.tile([P, SQ], F32, tag="st", name="pp_pre")[:, 0:512]
            # pre-stream copies ride the idle ACT engine (it has nothing to
            # do before the first exp; DVE would add queue latency)
            for _ in proj_qk("q", qT, 0, (0,), alloc=st_half, copy_eng=nc.scalar.copy):
                pass
            for _ in proj_qk("k", kT, 0, (0,), alloc=st_half, copy_eng=nc.scalar.copy):
                pass
            for _ in proj_qk("q", qT, 0, (1,), copy_eng=nc.scalar.copy):
                pass

            # Three priority queues, one task-step per slot: pv units on even
            # slots, projections on odd slots, out-proj fills leftover slots.
            # qkq order honors data deadlines: k m0 rest for head-0 blocks
            # 4..15, v chunks for the trailing PV, then later m chunks.
            vq = [
                proj_qk("k", kT, 0, (1,)),
                proj_v(0),
                proj_qk("k", kT, 0, (2,)),
                proj_v(1),
                proj_qk("k", kT, 0, (3,)),
                proj_v(2),
                proj_v(3),
            ]
            qkq = []
            for m in (1, 2, 3):
                qkq.append(proj_qk("q", qT, m, (0, 1)))
                qkq.append(proj_qk("k", kT, m, (0, 1, 2, 3)))
            pvq = []
            opq = []

            def step_q(q):
                while q:
                    if next(q[0], "done") == "done":
                        q.pop(0)
                    else:
                        return True
                return False

            def step_tasks():
                for q in (vq, pvq, qkq, opq):
                    if step_q(q):
                        return

            # ---------- attention slot stream ----------
            # pT ring: 24 tiles so exp(h+1, blk) only reuses a tile whose
            # PV reads (running during head h+1, one qc-unit per slot,
            # done by slot ~7) have completed.
            PT_BUFS = 32

            def pv_head(h, pTs):
                """PV + normalize for head h: per q-chunk, accumulate all 16
                key blocks in one PSUM bank (one zero region), then divide by
                the denominator column. For the last head, the head-pair-3
                transpose/out-proj steps are inlined per q-chunk to shorten
                the tail."""
                for qc in range(8):
                    pv = ps_pv.tile([P, 512], F32, tag="pv", name="pv")
                    for blk in range(NKB):
                        nc.tensor.matmul(
                            pv[:, 0:VW],
                            pTs[blk][:, qc * P : (qc + 1) * P],
                            v_aug[:, blk, h * VW : (h + 1) * VW],
                            start=(blk == 0),
                            stop=(blk == NKB - 1),
                        )
                    recip = wpool.tile([P, 1], F32, tag="recip", bufs=2)
                    nc.vector.reciprocal(recip, pv[:, DH : DH + 1])
                    if h == H - 1:
                        # ACT is idle post-stream; keep the tail off DVE
                        nc.scalar.mul(
                            att_n[:, qc, h * DH : (h + 1) * DH], pv[:, 0:DH], recip
                        )
                    else:
                        nc.vector.tensor_scalar_mul(
                            att_n[:, qc, h * DH : (h + 1) * DH], pv[:, 0:DH], recip
                        )
                    if h == H - 1:
                        for _ in outproj_nb(3, qc):
                            pass
                    yield
                if h % 2 == 1 and h < H - 1:
                    opq.append(outproj_hp(h // 2))

            for h in range(H):
                po = (h % 2) * DH
                mc = h // 2
                pTs = []
                for blk in range(NKB):
                    # scores + exp go first so a stalled task matmul (waiting
                    # on PSUM reuse) never blocks them in the in-order PE queue
                    st = ps_st.tile([P, SQ], F32, tag="st")
                    for nq in range(2):
                        nc.tensor.matmul(
                            st[:, nq * 512 : (nq + 1) * 512],
                            kT[po : po + DH, mc, blk * P : (blk + 1) * P],
                            qT[po : po + DH, mc, nq * 512 : (nq + 1) * 512],
                            start=True,
                            stop=True,
                        )
                    pT = wpool.tile([P, SQ], BF16, tag="pT", bufs=PT_BUFS)
                    if h == 0 and blk == 0:
                        # two half-width exps: the first only needs q chunk 0
                        for nq in range(2):
                            nc.scalar.activation(
                                pT[:, nq * 512 : (nq + 1) * 512],
                                st[:, nq * 512 : (nq + 1) * 512],
                                EXP,
                                scale=float(EXP_SCALE),
                            )
                    else:
                        nc.scalar.activation(pT, st, EXP, scale=float(EXP_SCALE))
                    pTs.append(pT)
                    if h == 0:
                        step_q(vq)
                    else:
                        step_tasks()
                pvq.append(pv_head(h, pTs))
            for q in (vq, pvq, opq, qkq):
                while step_q(q):
                    pass


_CACHED_NC = None


def _get_nc():
    global _CACHED_NC
    if _CACHED_NC is not None:
        return _CACHED_NC
    nc = bacc.Bacc("TRN2", target_bir_lowering=False, debug=False)
    def dt(name, shape, dtype):
        return nc.dram_tensor(name, shape, dtype, kind="ExternalInput").ap()
    xq = [dt(f"xq8_{i}", [P, 4, SQ], F8) for i in range(2)]
    xk = [dt(f"xk8_{i}", [P, 4, S], F8) for i in range(2)]
    xv = [dt(f"xv8_{i}", [P, 4, S], F8) for i in range(2)]
    w_ds = [dt(f"w8{w}{i}", [P, 4, D], F8) for w in "qkv" for i in range(2)]
    wo = dt("wo", [P, 4, D], BF16)
    iden = dt("iden", [P, P], BF16)
    out = nc.dram_tensor("out", [SQ, D], BF16, kind="ExternalOutput").ap()
    with tile.TileContext(nc) as tc:
        _build_mha(tc, out, xq, xk, xv, w_ds, wo, iden)
    nc.compile()
    return_nc = nc
    globals()["_CACHED_NC"] = return_nc
    return return_nc


def _cwise(xT):
    # [512, n] -> [128, 4, n] (d_in chunked)
    n = xT.shape[1]
    return np.ascontiguousarray(xT.reshape(4, P, n).transpose(1, 0, 2))


def _split8(a):
    hi = a.astype(FP8)
    lo = (a - hi.astype(np.float32)).astype(FP8)
    return hi, lo


def _w_arrange(w):
    return _cwise(np.asarray(w, np.float32) * WSCALE)


def _run(in_query, in_key, in_value, W_q, W_k, W_v, W_out, **run_kwargs):
    f = lambda a: np.asarray(a, dtype=np.float32)
    in_query, in_key, in_value = f(in_query), f(in_key), f(in_value)
    w_splits = []
    for w in (W_q, W_k, W_v):
        w_splits.extend(_split8(_w_arrange(w)))
    wo = _cwise(f(W_out)).astype(BF16NP)
    iden = np.eye(P, dtype=BF16NP)
    xk8 = [_split8(_cwise(in_key[b].T)) for b in range(B)]
    xv8 = [_split8(_cwise(in_value[b].T)) for b in range(B)]
    in_maps = []
    for c in range(NCORES):
        b, half = c // 2, c % 2
        xq_hi, xq_lo = _split8(_cwise(in_query[b, half * SQ : (half + 1) * SQ, :].T))
        im = {"xq8_0": xq_hi, "xq8_1": xq_lo,
              "xk8_0": xk8[b][0], "xk8_1": xk8[b][1],
              "xv8_0": xv8[b][0], "xv8_1": xv8[b][1],
              "wo": wo, "iden": iden}
        for wi, (w, i) in enumerate([(w, i) for w in "qkv" for i in range(2)]):
            im[f"w8{w}{i}"] = w_splits[wi]
        in_maps.append(im)
    res = run_bass_kernel_spmd(_get_nc(), in_maps, list(range(NCORES)), **run_kwargs)
    out = np.empty((B, S, D), np.float32)
    for c in range(NCORES):
        b, half = c // 2, c % 2
        out[b, half * SQ : (half + 1) * SQ, :] = res.results[c]["out"].astype(
            np.float32
        )
    return out, res


def kernel(in_query, in_key, in_value, W_q, W_k, W_v, W_out):
    out, _ = _run(in_query, in_key, in_value, W_q, W_k, W_v, W_out)
    return out
